# revision 35
# baseline (speedup 1.0000x reference)
"""Trainium2 Bass kernel for a transformer encoder layer (B=2, S=2048,
D=1024, H=16, FFN=4096), sharded over 8 NeuronCores.

Sharding: token-parallel. Cores 0-3 process batch 0, cores 4-7 batch 1;
each core owns a 512-token query window and computes the full layer for
those tokens. K/V are computed per-core for the whole batch (duplicated
across the 4 cores of a batch group) — no collectives.

Layout: activations are feature-major ("transposed", [d, token]) so all
matmuls chain without transposes. Attention scores are computed
transposed ([kv, q]); softmax runs without max-subtraction (scores are
O(1) for this input distribution; pad keys get a -30000 bias so exp
underflows to exactly 0). The softmax denominator comes from an appended
ones-column in V; per-query normalization broadcasts reciprocals across
partitions with a K=1 PE matmul.

Masked keys are compacted away on the host: positions with mask==1
contribute exactly 0 to numerator and denominator, so only unmasked
positions are projected/attended (~half of S).

Matmul chain runs in bf16 (weights + activations); residual adds,
layernorm statistics and softmax denominators stay in fp32/fp32r.

Scheduling notes (vs the first working version):
- weights/activations stream in a few large packed DMAs ordered so the
  first Q matmul starts ~3us in; ones tiles come from memset, not DMA.
- layernorm statistics interleave with their producers (Wo / W2), the
  sqrt activation table is preloaded during attention's tail, the
  mean/var chain is fused, and the normalize applies alpha via
  scalar_tensor_tensor with beta folded into a PE-broadcast tensor.
- the LN2 apply is split across DVE and GpSimd (Pool) with the final
  scale on ACT for the Pool half, to shorten the kernel tail.
"""

from contextlib import ExitStack

import ml_dtypes
import numpy as np

import concourse.bass as bass  # noqa: F401
import concourse.mybir as mybir
import concourse.tile as tile
from concourse import bacc
from concourse.bass_utils import run_bass_kernel_spmd

f32 = mybir.dt.float32
f32r = mybir.dt.float32r
bf16 = mybir.dt.bfloat16
i16 = mybir.dt.int16
AF = mybir.ActivationFunctionType
ALU = mybir.AluOpType

# Schraudolph fast-exp constants, bf16-bits variant:
# bf16 shares f32's exponent layout with a 7-bit mantissa, so
# exp(x) ~= bitcast_bf16(int16(A16*x + B16)), A16 = 2^7/log(2).
# A16 folds the 0.125 score scale; the -0.7*2^7/... shift centers the
# piecewise-linear error (classic magic-constant offset scaled to 2^7).
SCH_A16 = (128.0 / 0.6931471805599453) * 0.125
SCH_B16 = 16249.0

D = 1024
H = 16
DEP = 64
HID = 4096
B = 2
S = 2048
QLOC = 512
NCORES = 8
PADBIAS = -30000.0

P = 128
KT_D = D // P
MT_D = D // P
MT_H = HID // P
NPAIR = H // 2
VW = DEP + 1

PHASES = {}


def _mark(nc, name):
    PHASES[name] = nc.next_id()


# cpack column layout (f32, [P, CW]): widths per field
CFIELDS = [("bq", MT_D), ("bk", MT_D), ("bo", MT_D), ("b1", MT_H),
           ("b2", MT_D), ("a1", MT_D), ("be1", MT_D), ("a2", MT_D),
           ("be2", MT_D)]


def build(nkv: int, dve_ti: tuple = ()):
    assert nkv % P == 0
    nkt = nkv // P
    nchunk = (nkv + 511) // 512  # kv chunks of <=512

    cw = sum(w for _, w in CFIELDS) + nkt
    coff = {}
    off = 0
    for nm, w in CFIELDS:
        coff[nm] = off
        off += w
    coff["mb"] = off

    nc = bacc.Bacc(None, target_bir_lowering=False, debug=False)

    xq_d = nc.dram_tensor("xq", [P, D // P * QLOC], bf16, kind="ExternalInput")
    # xkv packed per kv-chunk: [P, nchunk * KT_D * chunkw]
    xkv_d = nc.dram_tensor("xkv", [P, KT_D * nkv], bf16, kind="ExternalInput")
    cpack_d = nc.dram_tensor("cpack", [P, cw], f32, kind="ExternalInput")
    wq_d = nc.dram_tensor("wq", [P, KT_D * D], bf16, kind="ExternalInput")
    wk_d = nc.dram_tensor("wk", [P, KT_D * D], bf16, kind="ExternalInput")
    wv_d = nc.dram_tensor("wv", [P, KT_D * D], bf16, kind="ExternalInput")
    wo_d = nc.dram_tensor("wo", [P, KT_D * D], bf16, kind="ExternalInput")
    w1_d = nc.dram_tensor("w1", [P, KT_D * HID], bf16, kind="ExternalInput")
    w2_d = nc.dram_tensor("w2", [P, MT_H * D], bf16, kind="ExternalInput")
    out_d = nc.dram_tensor("out", [D, QLOC], f32, kind="ExternalOutput")

    with tile.TileContext(nc) as tc, \
         nc.allow_low_precision(reason="bf16/f32r matmul inputs"), \
         ExitStack() as ctx:
        # ---- constants ----
        cst = ctx.enter_context(tc.tile_pool(name="cst", bufs=1))
        cpk = cst.tile([P, cw], f32)
        nc.sync.dma_start(out=cpk[:], in_=cpack_d[:])

        def ccol(nm, m=None, w=1):
            o = coff[nm]
            if m is None:
                return cpk[:, o:o + dict(CFIELDS)[nm]]
            return cpk[:, o + m:o + m + w]

        mbias = cpk[:, coff["mb"]:coff["mb"] + nkt]

        # ones tiles via memset + f32r rounding copy (no DMA)
        ones_f = cst.tile([P, 1], f32)
        nc.vector.memset(ones_f[:], 1.0)
        ones = cst.tile([P, 1], f32r)          # column of ones (LN sums lhsT)
        nc.vector.tensor_copy(ones[:], ones_f[:])
        scr = cst.tile([1, 1], f32)           # ACT table preload scratch
        # preload the exp table while the pipe is otherwise empty
        nc.scalar.activation(scr[:], cpk[0:1, 0:1], AF.Exp)

        # ---- x loads (ordered for fast Q start) ----
        es_x = ExitStack()
        p_xq = es_x.enter_context(tc.tile_pool(name="p_xq", bufs=1, side="right"))
        xqp = p_xq.tile([P, KT_D * QLOC], bf16, name="xqp")
        nc.sync.dma_start(out=xqp[:, 0:QLOC], in_=xq_d[:, 0:QLOC])
        xq = [xqp[:, k * QLOC:(k + 1) * QLOC] for k in range(KT_D)]

        es_w = ExitStack()
        wpool = es_w.enter_context(tc.tile_pool(name="wpool", bufs=2, side="right"))

        def load_whalf(dram, nm, half, split=False):
            t = wpool.tile([P, KT_D * 512], bf16, name=f"{nm}{half}", tag="w")
            base = half * 4096
            if split:
                nc.sync.dma_start(out=t[:, 0:512], in_=dram[:, base:base + 512])
                nc.sync.dma_start(out=t[:, 512:], in_=dram[:, base + 512:base + 4096])
            else:
                nc.sync.dma_start(out=t[:], in_=dram[:, base:base + 4096])
            return t

        # pools (left-SBUF creation order fixes LIFO release order)
        p_kt = ctx.enter_context(tc.tile_pool(name="p_kt", bufs=MT_D))
        p_qr = ctx.enter_context(tc.tile_pool(name="p_qr", bufs=MT_D))
        p_z = ctx.enter_context(tc.tile_pool(name="p_z", bufs=MT_D))
        w1p = ctx.enter_context(tc.tile_pool(name="w1p", bufs=2))
        ln_s = ctx.enter_context(tc.tile_pool(name="ln_s", bufs=2))
        es_attnT = ExitStack()
        p_attnT = es_attnT.enter_context(tc.tile_pool(name="p_attnT", bufs=MT_D))
        es_vaug = ExitStack()
        p_vaug = es_vaug.enter_context(tc.tile_pool(name="p_vaug", bufs=nkt))
        es_kv = ExitStack()
        p_xkv = es_kv.enter_context(tc.tile_pool(name="p_xkv", bufs=1))
        xkvp = p_xkv.tile([P, KT_D * nkv], bf16, name="xkvp")
        es_pp1 = ExitStack()
        pp1 = es_pp1.enter_context(
            tc.tile_pool(name="pp1", bufs=4, space="PSUM", side="right"))

        # ---- Q^T ----
        _mark(nc, 'qt')
        wq0 = load_whalf(wq_d, "wq", 0, split=True)
        nc.sync.dma_start(out=xqp[:, QLOC:], in_=xq_d[:, QLOC:])
        wq1 = load_whalf(wq_d, "wq", 1)
        xkv = [xkvp[:, k * nkv:(k + 1) * nkv] for k in range(KT_D)]

        qt = []
        for half in range(2):
            wq = wq0 if half == 0 else wq1
            pss = [pp1.tile([P, QLOC], f32, name=f"qt_ps{ml}", tag="ps")
                   for ml in range(4)]
            for k in range(KT_D):
                for ml in range(4):
                    nc.tensor.matmul(pss[ml][:],
                                     wq[:, k * 512 + ml * P:k * 512 + (ml + 1) * P],
                                     xq[k],
                                     start=(k == 0), stop=(k == KT_D - 1))
            for ml in range(4):
                m = half * 4 + ml
                t = p_qr.tile([P, QLOC], bf16, name=f"qt{m}", tag="qr")
                nc.vector.tensor_scalar_add(t[:], pss[ml][:], ccol("bq", m))
                qt.append(t)

        # ---- K^T ----
        _mark(nc, 'kt')
        wk0 = load_whalf(wk_d, "wk", 0)
        wk1 = load_whalf(wk_d, "wk", 1)
        # xkv chunk loads: chunk 0 lands before the first K matmul needs it
        for c in range(nchunk):
            cwid = min(512, nkv - c * 512)
            nc.sync.dma_start(
                out=xkvp[:].rearrange("p (k c) -> p k c", c=nkv)[
                    :, :, c * 512:c * 512 + cwid],
                in_=xkv_d[:].rearrange("p (k c) -> p k c", c=nkv)[
                    :, :, c * 512:c * 512 + cwid])
        kt = []
        for half in range(2):
            wk = wk0 if half == 0 else wk1
            for ml in range(4):
                m = half * 4 + ml
                t = p_kt.tile([P, nkv], bf16, name=f"kt{m}", tag="kt")
                for c in range(nchunk):
                    off = c * 512
                    cwid = min(512, nkv - off)
                    ps = pp1.tile([P, 512], f32, name="kt_ps", tag="ps")
                    for k in range(KT_D):
                        nc.tensor.matmul(
                            ps[:, :cwid],
                            wk[:, k * 512 + ml * P:k * 512 + (ml + 1) * P],
                            xkv[k][:, off:off + cwid],
                            start=(k == 0), stop=(k == KT_D - 1))
                    nc.vector.tensor_scalar_add(t[:, off:off + cwid],
                                                ps[:, :cwid], ccol("bk", m))
                kt.append(t)

        # ---- V (token-major) with interleaved per-head ones column ----
        _mark(nc, 'v')
        wv0 = load_whalf(wv_d, "wv", 0)
        wv1 = load_whalf(wv_d, "wv", 1)
        vaug = []
        for ti in range(nkt):
            t = p_vaug.tile([P, H * VW], bf16, name=f"vaug{ti}", tag="vaug")
            v3 = t[:].rearrange("p (h c) -> p h c", c=VW)
            nc.gpsimd.memset(v3[:, :, DEP], 1.0)
            vaug.append(t)
        for half in range(2):
            wv = wv0 if half == 0 else wv1
            for ti in range(nkt):
                ps = pp1.tile([P, 512], f32, name="v_ps", tag="ps")
                for k in range(KT_D):
                    nc.tensor.matmul(
                        ps[:], xkv[k][:, ti * P:(ti + 1) * P],
                        wv[:, k * 512:(k + 1) * 512],
                        start=(k == 0), stop=(k == KT_D - 1))
                v3 = vaug[ti][:].rearrange("p (h c) -> p h c", c=VW)
                dst = v3[:, half * 8:(half + 1) * 8, 0:DEP]
                vsrc = ps[:].rearrange("p (h c) -> p h c", c=DEP)
                nc.scalar.copy(dst, vsrc)
        es_kv.close()
        es_pp1.close()
        # V copies ran on ACT (Identity); swap the exp table back in
        # before the attention exps start
        nc.scalar.activation(scr[:], cpk[0:1, 0:1], AF.Exp)

        # ---- attention ----
        # The per-pair normalize (recip -> Pool partition-broadcast ->
        # multiply) is software-pipelined into the NEXT pair's kv loop so
        # PE never waits on it. op pool holds two pairs of output psums.
        _mark(nc, 'attn')
        ep = ExitStack()
        ep_sp = ExitStack()
        epl = ep.enter_context(tc.tile_pool(name="epl", bufs=3, side="right"))
        nrm = ep.enter_context(tc.tile_pool(name="nrm", bufs=2, side="right"))
        op = ep.enter_context(tc.tile_pool(name="op", bufs=2, space="PSUM"))
        sp = ep_sp.enter_context(tc.tile_pool(name="sp", bufs=3, space="PSUM"))
        attnT = [None] * NPAIR

        def fin_recs(st):
            st["recA"] = nrm.tile([1, QLOC], f32, name="recA", tag="rec")
            st["recB"] = nrm.tile([1, QLOC], f32, name="recB", tag="rec")
            nc.vector.reciprocal(st["recA"][:], st["psoA"][DEP:VW, :])
            nc.vector.reciprocal(st["recB"][:], st["psoB"][DEP:VW, :])

        def fin_bcast(st):
            st["rbA"] = nrm.tile([DEP, QLOC], f32, name="rbA", tag="rb")
            st["rbB"] = nrm.tile([DEP, QLOC], f32, name="rbB", tag="rb")
            nc.gpsimd.partition_broadcast(st["rbA"][:], st["recA"][:])
            nc.gpsimd.partition_broadcast(st["rbB"][:], st["recB"][:])

        def fin_mul(st):
            hp = st["hp"]
            at = p_attnT.tile([P, QLOC], bf16, name=f"attnT{hp}", tag="attnT")
            nc.vector.tensor_mul(at[0:DEP, :], st["psoA"][0:DEP, :],
                                 st["rbA"][:])
            tmpB = nrm.tile([DEP, QLOC], bf16, name="tmpB", tag="tmpB")
            nc.vector.tensor_mul(tmpB[:], st["psoB"][0:DEP, :], st["rbB"][:])
            nc.sync.dma_start(out=at[DEP:P, :], in_=tmpB[:])
            attnT[hp] = at

        prev = None
        for hp in range(NPAIR):
            hA, hB = 2 * hp, 2 * hp + 1
            psoA = op.tile([P, QLOC], f32, name="psoA", tag="pso")
            psoB = op.tile([P, QLOC], f32, name="psoB", tag="pso")

            def scores(ti):
                kvs = slice(ti * P, (ti + 1) * P)
                psAB = sp.tile([P, 2 * QLOC], f32, name="psAB", tag="sc")
                nc.tensor.matmul(psAB[:, 0:QLOC], kt[hp][0:DEP, kvs],
                                 qt[hp][0:DEP, :],
                                 start=True, stop=True, tile_position=(0, 0))
                nc.tensor.matmul(psAB[:, QLOC:2 * QLOC], kt[hp][DEP:P, kvs],
                                 qt[hp][DEP:P, :],
                                 start=True, stop=True, tile_position=(64, 0))
                if ti in dve_ti:
                    # Schraudolph fast exp on DVE, bf16-bits variant:
                    # bf16(exp(x)) ~= bitcast_bf16(int16(A16*x + B16))
                    ei = epl.tile([P, 2 * QLOC], i16, name="eABi", tag="e")
                    nc.vector.tensor_scalar(ei[:], psAB[:], SCH_A16, SCH_B16,
                                            ALU.mult, ALU.add)
                    return ("i", ei)
                eAB = epl.tile([P, 2 * QLOC], bf16, name="eAB", tag="e")
                nc.scalar.activation(eAB[:], psAB[:], AF.Exp,
                                     bias=mbias[:, ti:ti + 1], scale=0.125)
                return ("b", eAB)

            def eslice(e, lo, hi):
                tag, t = e
                ap = t[:, lo:hi]
                return ap.bitcast(bf16) if tag == "i" else ap

            eAB = scores(0)
            if prev is not None:
                fin_recs(prev)
            for ti in range(nkt):
                nxt = scores(ti + 1) if ti + 1 < nkt else None
                st, fi = (ti == 0), (ti == nkt - 1)
                nc.tensor.matmul(psoA[0:VW, :], vaug[ti][:, hA * VW:(hA + 1) * VW],
                                 eslice(eAB, 0, QLOC), start=st, stop=fi)
                nc.tensor.matmul(psoB[0:VW, :], vaug[ti][:, hB * VW:(hB + 1) * VW],
                                 eslice(eAB, QLOC, 2 * QLOC), start=st, stop=fi)
                if prev is not None:
                    if ti == 0:
                        fin_bcast(prev)
                    elif ti == nkt - 3:
                        fin_mul(prev)
                eAB = nxt
            prev = {"hp": hp, "psoA": psoA, "psoB": psoB}
            if hp == 0:
                # w1 group-0 preload rides under attention
                t = w1p.tile([P, KT_D * 1024], bf16, name="w1g0", tag="w1")
                nc.sync.dma_start(out=t[:], in_=w1_d[:, 0:KT_D * 1024])
                w1g_tiles = [t]
        ep_sp.close()  # free the scores psum banks before the normalize tail
        fin_recs(prev)
        fin_bcast(prev)
        fin_mul(prev)
        ep.close()
        es_vaug.close()

        # preload the sqrt table while ACT is otherwise idle (post-exp)
        nc.scalar.activation(scr[:], cpk[0:1, 0:1], AF.Sqrt)

        # ---- Wo + residual + interleaved LN1 stats ----
        _mark(nc, 'wo')
        wo0 = load_whalf(wo_d, "wo", 0)
        wo1 = load_whalf(wo_d, "wo", 1)
        pp2 = ctx.enter_context(
            tc.tile_pool(name="pp2", bufs=2, space="PSUM", side="right"))
        lnp = ctx.enter_context(
            tc.tile_pool(name="lnp", bufs=2, space="PSUM", side="right"))
        es_w1pp = ExitStack()
        w1pp = es_w1pp.enter_context(
            tc.tile_pool(name="w1pp", bufs=4, space="PSUM", side="right"))
        ssum1 = lnp.tile([1, QLOC], f32, name="ssum1", tag="lnps")
        ssq1 = lnp.tile([1, QLOC], f32, name="ssq1", tag="lnps")
        r1 = []
        for half in range(2):
            wo = wo0 if half == 0 else wo1
            for ml in range(4):
                m = half * 4 + ml
                ps = w1pp.tile([P, QLOC], f32, name="wo_ps", tag="w1ps")
                for k in range(KT_D):
                    nc.tensor.matmul(
                        ps[:], wo[:, k * 512 + ml * P:k * 512 + (ml + 1) * P],
                        attnT[k][:],
                        start=(k == 0), stop=(k == KT_D - 1))
                t = p_qr.tile([P, QLOC], f32r, name=f"r1_{m}", tag="qr")
                nc.vector.scalar_tensor_tensor(
                    t[:], ps[:], ccol("bo", m),
                    xqp[:, m * QLOC:(m + 1) * QLOC], ALU.add, ALU.add)
                r1.append(t)
                nc.tensor.matmul(ssum1[:], ones[:, 0:1], t[:],
                                 start=(m == 0), stop=(m == MT_D - 1))
                sq = ln_s.tile([P, QLOC], f32r, name="sq1", tag="sq", bufs=2)
                nc.vector.tensor_mul(sq[:], t[:].bitcast(f32),
                                     t[:].bitcast(f32))
                nc.tensor.matmul(ssq1[:], ones[:, 0:1], sq[:],
                                 start=(m == 0), stop=(m == MT_D - 1))
        es_w.close()
        es_x.close()
        es_attnT.close()

        def ln_head(ssum, ssq, tag):
            """Fused mean/var chain: returns (rstd, mrs) [1,QLOC] f32r."""
            n = D
            s1 = ln_s.tile([1, QLOC], f32, name=f"s1{tag}", tag="lns", bufs=7)
            nc.vector.tensor_copy(s1[:], ssum[:])
            t = ln_s.tile([1, QLOC], f32, name=f"t{tag}", tag="lns", bufs=7)
            nc.vector.scalar_tensor_tensor(t[:], s1[:], 1.0 / n, s1[:],
                                           ALU.mult, ALU.mult)
            vr = ln_s.tile([1, QLOC], f32, name=f"vr{tag}", tag="lns", bufs=7)
            nc.vector.tensor_sub(vr[:], ssq[:], t[:])
            std = ln_s.tile([1, QLOC], f32, name=f"std{tag}", tag="lns", bufs=7)
            nc.scalar.activation(std[:], vr[:], AF.Sqrt, scale=1.0 / (n - 1))
            rstd = ln_s.tile([1, QLOC], f32r, name=f"rstd{tag}", tag="lns", bufs=7)
            nc.vector.reciprocal(rstd[:], std[:])
            mrs = ln_s.tile([1, QLOC], f32r, name=f"mrs{tag}", tag="lns", bufs=7)
            nc.vector.scalar_tensor_tensor(mrs[:], s1[:], 1.0 / n,
                                           rstd[:].bitcast(f32),
                                           ALU.mult, ALU.mult)
            return rstd, mrs

        # ---- LN1 (normalize on DVE/Pool, alpha/beta on ACT; the rstd and
        # mean*rstd rows are partition-broadcast on Pool, no PSUM needed) ----
        _mark(nc, 'ln1')
        rstd1, mrs1 = ln_head(ssum1, ssq1, "1")
        out1 = [None] * MT_D
        bcs1 = ln_s.tile([P, 2 * QLOC], f32, name="bcs1", tag="lnb")
        rsb1s = bcs1[:, 0:QLOC]
        m2bs1 = bcs1[:, QLOC:]
        nc.gpsimd.partition_broadcast(rsb1s, rstd1[:].bitcast(f32))
        nc.gpsimd.partition_broadcast(m2bs1, mrs1[:].bitcast(f32))
        z1 = [None] * MT_D
        for m in range(MT_D):
            z = p_z.tile([P, QLOC], bf16, name=f"z1_{m}", tag="z")
            o = p_qr.tile([P, QLOC], f32, name=f"out1_{m}", tag="qr")
            eng = nc.gpsimd if m in (1, 3, 5, 7) else nc.vector
            tm = ln_s.tile([P, QLOC], f32, name="tm1", tag="tm", bufs=3)
            eng.tensor_mul(tm[:], r1[m][:].bitcast(f32), rsb1s)
            nc.vector.tensor_sub(z[:], tm[:], m2bs1)
            nc.scalar.activation(o[:], z[:], AF.Identity,
                                 bias=ccol("be1", m), scale=ccol("a1", m))
            z1[m] = z
            out1[m] = o

        def o1r(k):
            return z1[k][:]

        # ---- FFN first linear ----
        _mark(nc, 'w1')
        p_ht = ctx.enter_context(tc.tile_pool(name="p_ht", bufs=MT_H))
        ht = []
        # group 0 runs k-major so PE starts as soon as out1[k] tiles land
        for rnd in range(2):
            pss = [w1pp.tile([P, QLOC], f32, name=f"w1ps{rnd}_{mi}",
                             tag="w1ps") for mi in range(4)]
            for k in range(KT_D):
                for mi in range(4):
                    mm = rnd * 4 + mi
                    nc.tensor.matmul(
                        pss[mi][:],
                        w1g_tiles[0][:, k * 1024 + mm * P:
                                     k * 1024 + (mm + 1) * P],
                        o1r(k), start=(k == 0), stop=(k == KT_D - 1))
            for mi in range(4):
                mm = rnd * 4 + mi
                t = p_ht.tile([P, QLOC], bf16, name=f"ht{mm}", tag="ht")
                nc.scalar.activation(t[:], pss[mi][:], AF.Relu,
                                     bias=ccol("b1", mm))
                ht.append(t)
        es_w1pp.close()
        for g in range(1, 4):
            w1g = w1p.tile([P, KT_D * 1024], bf16, name=f"w1g{g}", tag="w1")
            nc.sync.dma_start(
                out=w1g[:], in_=w1_d[:, g * KT_D * 1024:(g + 1) * KT_D * 1024])
            for mm in range(8):
                m = g * 8 + mm
                ps = pp2.tile([P, QLOC], f32, name="h_ps", tag="ps2")
                for k in range(KT_D):
                    nc.tensor.matmul(
                        ps[:],
                        w1g[:, k * 1024 + mm * P:k * 1024 + (mm + 1) * P],
                        o1r(k),
                        start=(k == 0), stop=(k == KT_D - 1))
                t = p_ht.tile([P, QLOC], bf16, name=f"ht{m}", tag="ht")
                nc.scalar.activation(t[:], ps[:], AF.Relu,
                                     bias=ccol("b1", m))
                ht.append(t)

        # ---- FFN second linear + interleaved LN2 stats ----
        _mark(nc, 'w2')
        ssum2 = lnp.tile([1, QLOC], f32, name="ssum2", tag="lnps")
        ssq2 = lnp.tile([1, QLOC], f32, name="ssq2", tag="lnps")
        r2 = []
        w2p = ctx.enter_context(tc.tile_pool(name="w2p", bufs=2, side="right"))
        KH = MT_H // 2
        with tc.tile_pool(name="fpp", bufs=1, space="PSUM", side="right") as fpp:
            for mg in range(2):
                w2t = []
                for kh in range(2):
                    t = w2p.tile([P, KH * 512], bf16, name=f"w2q{mg}{kh}",
                                 tag="w2")
                    base = mg * MT_H * 512 + kh * KH * 512
                    nc.sync.dma_start(out=t[:],
                                      in_=w2_d[:, base:base + KH * 512])
                    w2t.append(t)
                f_ps = [fpp.tile([P, QLOC], f32, name=f"f_ps{mg}_{m}",
                                 tag=f"fps{m}", bufs=1) for m in range(4)]
                for k in range(MT_H):
                    wt = w2t[k // KH]
                    kk = k % KH
                    for m in range(4):
                        nc.tensor.matmul(
                            f_ps[m][:],
                            wt[:, kk * 512 + m * P:kk * 512 + (m + 1) * P],
                            ht[k][:],
                            start=(k == 0), stop=(k == MT_H - 1))
                for m in range(4):
                    mi = mg * 4 + m
                    t = p_kt.tile([P, QLOC], f32r, name=f"r2_{mi}", tag="kt")
                    nc.vector.scalar_tensor_tensor(t[:], f_ps[m][:],
                                                   ccol("b2", mi),
                                                   out1[mi][:], ALU.add, ALU.add)
                    r2.append(t)
                    nc.tensor.matmul(ssum2[:], ones[:, 0:1], t[:],
                                     start=(mi == 0), stop=(mi == MT_D - 1))
                    sq = ln_s.tile([P, QLOC], f32r, name="sq2", tag="sq", bufs=2)
                    nc.vector.tensor_mul(sq[:], t[:].bitcast(f32),
                                         t[:].bitcast(f32))
                    nc.tensor.matmul(ssq2[:], ones[:, 0:1], sq[:],
                                     start=(mi == 0), stop=(mi == MT_D - 1))

        # ---- LN2: normalize on DVE/Pool, alpha/beta on ACT, DMA per m ----
        _mark(nc, 'ln2')
        rstd2, mrs2 = ln_head(ssum2, ssq2, "2")
        bcs2 = ln_s.tile([P, 2 * QLOC], f32, name="bcs2", tag="lnb")
        rsb2s = bcs2[:, 0:QLOC]
        m2bs = bcs2[:, QLOC:]
        nc.gpsimd.partition_broadcast(rsb2s, rstd2[:].bitcast(f32))
        nc.gpsimd.partition_broadcast(m2bs, mrs2[:].bitcast(f32))
        for m in range(MT_D):
            o = ln_s.tile([P, QLOC], f32, name=f"ln2_{m}", tag="o2",
                          bufs=3)
            eng = nc.gpsimd if m in (3, 7) else nc.vector
            tm = ln_s.tile([P, QLOC], f32, name="tm2", tag="tm", bufs=3)
            eng.tensor_mul(tm[:], r2[m][:].bitcast(f32), rsb2s)
            tm2 = ln_s.tile([P, QLOC], f32, name="tq2", tag="tq", bufs=3)
            eng.tensor_sub(tm2[:], tm[:], m2bs)
            nc.scalar.activation(o[:], tm2[:], AF.Identity,
                                 bias=ccol("be2", m), scale=ccol("a2", m))
            nc.sync.dma_start(out=out_d[m * P:(m + 1) * P, :], in_=o[:])
        _mark(nc, 'end')

    nc.compile()
    return nc


_cache = {}


def _get_nc(nkv, dve_ti=()):
    key = (nkv, dve_ti)
    if key not in _cache:
        _cache[key] = build(nkv, dve_ti)
    return _cache[key]


def _pack_w(w, ncolblk):
    """[R, C] -> [128, (R//128)*C] with k-tiles of 128 rows as col blocks."""
    r, c = w.shape
    kt = r // P
    return np.ascontiguousarray(
        w.reshape(kt, P, c).transpose(1, 0, 2).reshape(P, kt * c))


def kernel(x, mask, Wq, bq, Wk, bk, Wv, bv, Wo, bo, alpha1, beta1,
           W1, b1, W2, b2, alpha2, beta2):
    x = np.asarray(x, np.float32)
    mask = np.asarray(mask)

    idx = [np.nonzero(np.asarray(mask[b]) == 0)[0] for b in range(B)]
    nkv = ((max(len(i) for i in idx) + P - 1) // P) * P
    nkv = max(nkv, P)
    nkt = nkv // P

    # kv tiles that are pad-free for every batch may use the DVE fast-exp;
    # interleave them (odd indices) so ACT and DVE exps overlap instead of
    # serializing in blocks on the scores-psum rotation.
    safe = min(min(len(i) for i in idx) // P, nkt)
    nd = min(nkt // 3, safe)
    dve_ti = tuple(range(safe - nd, safe))

    nc = _get_nc(nkv, dve_ti)

    def colmaj(v):
        v = np.asarray(v, np.float32)
        return v.reshape(-1, P).T

    bo_eff = (np.asarray(bo, np.float32)
              + np.asarray(bv, np.float32) @ np.asarray(Wo, np.float32))

    # LN1's alpha folds into W1 rows, beta into b1: the kernel feeds W1 the
    # pre-affine normalized activations.
    W1 = np.asarray(W1, np.float32)
    W1_eff = np.asarray(alpha1, np.float32)[:, None] * W1
    b1_eff = np.asarray(b1, np.float32) + np.asarray(beta1, np.float32) @ W1

    bf = ml_dtypes.bfloat16

    # packed constants
    fields = {"bq": colmaj(bq), "bk": colmaj(bk), "bo": colmaj(bo_eff),
              "b1": colmaj(b1_eff), "b2": colmaj(b2), "a1": colmaj(alpha1),
              "be1": colmaj(beta1), "a2": colmaj(alpha2), "be2": colmaj(beta2)}
    cw = sum(w for _, w in CFIELDS) + nkt

    # w2 packed per mg: [4096, 1024] -> mg slices of 512 cols, k-tiles packed
    W2f = np.asarray(W2, bf)
    w2pack = np.concatenate(
        [_pack_w(np.ascontiguousarray(W2f[:, mg * 512:(mg + 1) * 512]), 512)
         for mg in range(2)], axis=1)

    def _pack_blk(w, nblk, blkw):
        # [R, nblk*blkw] -> [128, nblk * (R//128) * blkw]:
        # layout [p, b*kt*blkw + k*blkw + col] = w[k*128+p, b*blkw+col]
        r = w.shape[0]
        kt = r // P
        return np.ascontiguousarray(
            w.reshape(kt, P, nblk, blkw).transpose(1, 2, 0, 3)
            .reshape(P, nblk * kt * blkw))

    common = {
        "wq": _pack_blk(np.asarray(Wq, bf), 2, 512),
        "wk": _pack_blk(np.asarray(Wk, bf), 2, 512),
        "wv": _pack_blk(np.asarray(Wv, bf), 2, 512),
        "wo": _pack_blk(np.asarray(Wo, bf), 2, 512),
        "w1": _pack_blk(np.asarray(W1_eff, bf), 4, 1024),
        "w2": w2pack,
    }

    per_batch = []
    for b in range(B):
        ib = idx[b]
        xkv = np.zeros((D, nkv), bf)
        xkv[:, :len(ib)] = x[b][ib].T.astype(bf)
        mb = np.zeros(nkv, np.float32)
        mb[len(ib):] = PADBIAS
        mb = np.ascontiguousarray(mb.reshape(nkt, P).T)
        cpk = np.zeros((P, cw), np.float32)
        off = 0
        for nm, w in CFIELDS:
            cpk[:, off:off + w] = fields[nm]
            off += w
        cpk[:, off:off + nkt] = mb
        per_batch.append((_pack_w(xkv, nkv), np.ascontiguousarray(cpk),
                          np.ascontiguousarray(x[b].T)))

    in_maps = []
    for c in range(NCORES):
        b = c // 4
        qoff = (c % 4) * QLOC
        xkvp, cpk, xT = per_batch[b]
        xq_blk = xT[:, qoff:qoff + QLOC]
        m = dict(common)
        m["xq"] = _pack_w(np.ascontiguousarray(xq_blk.astype(bf)), QLOC)
        m["xkv"] = xkvp
        m["cpack"] = cpk
        in_maps.append(m)

    res = None
    for attempt in range(3):
        try:
            res = run_bass_kernel_spmd(nc, in_maps, list(range(NCORES)))
            break
        except Exception:
            if attempt == 2:
                raise

    out = np.empty((B, S, D), np.float32)
    for c in range(NCORES):
        b = c // 4
        qoff = (c % 4) * QLOC
        out[b, qoff:qoff + QLOC, :] = res.results[c]["out"].T
    return out



# revision 39
# speedup vs baseline: 1.1298x; 1.1298x over previous
"""Trainium2 Bass kernel for a transformer encoder layer (B=2, S=2048,
D=1024, H=16, FFN=4096), sharded over 8 NeuronCores.

Sharding: token-parallel. Cores 0-3 process batch 0, cores 4-7 batch 1;
each core owns a 512-token query window and computes the full layer for
those tokens. K/V are computed per-core for the whole batch (duplicated
across the 4 cores of a batch group) — no collectives.

Layout: activations are feature-major ("transposed", [d, token]) so all
matmuls chain without transposes. Attention scores are computed
transposed ([kv, q]); softmax runs without max-subtraction (scores are
O(1) for this input distribution; pad keys get a -30000 bias so exp
underflows to exactly 0). The softmax denominator comes from an appended
ones-column in V; per-query normalization broadcasts reciprocals across
partitions with a K=1 PE matmul.

Masked keys are compacted away on the host: positions with mask==1
contribute exactly 0 to numerator and denominator, so only unmasked
positions are projected/attended (~half of S).

Precision: Q/K/V projections run fp8e4 DoubleRow (2x PE) with weights
pre-scaled x64 on the host (the 1/64 is folded into the bias ops); the
fp8 quantization only touches the attention path, whose contribution to
the output is small. Wo/W1/W2 run bf16. Residual adds, layernorm
statistics and softmax denominators stay in fp32/fp32r. A third of each
head-pair's softmax exp tiles run on DVE via a Schraudolph fast-exp in
bf16-bits (int16 linear map + bitcast); the rest stay on ACT's exact
exp table, balancing the two engines.

Scheduling notes:
- weights/activations stream in a few large packed DMAs ordered so the
  first Q matmul starts ~3us in; ones tiles come from memset, not DMA.
- the per-pair softmax normalize (reciprocal -> Pool partition
  broadcast -> multiply) is software-pipelined into the next pair's kv
  loop; the scores psum pool triple-buffers so consecutive exps overlap
  across ACT/DVE; the scores pool closes before the last pair's
  normalize so Wo's psum pools don't wait on the attention tail.
- layernorm statistics interleave with their producers (Wo / W2); the
  apply normalizes on DVE/Pool into a bf16 pre-affine tensor that W1
  consumes directly (LN1's alpha/beta are folded into W1/b1 on the
  host); the residual copy gets alpha/beta on ACT off the critical
  path. rstd/mean broadcasts use Pool partition_broadcast, no PSUM.
- W1's first group runs k-major over 4 psum banks so PE starts as soon
  as each LN1 output tile lands.
"""

from contextlib import ExitStack

import ml_dtypes
import numpy as np

import concourse.bass as bass  # noqa: F401
import concourse.mybir as mybir
import concourse.tile as tile
from concourse import bacc
from concourse.bass_utils import run_bass_kernel_spmd

f32 = mybir.dt.float32
f32r = mybir.dt.float32r
bf16 = mybir.dt.bfloat16
i16 = mybir.dt.int16
fp8 = mybir.dt.float8e4
DR = mybir.MatmulPerfMode.DoubleRow
WSCALE = 64.0
AF = mybir.ActivationFunctionType
ALU = mybir.AluOpType

# Schraudolph fast-exp constants, bf16-bits variant:
# bf16 shares f32's exponent layout with a 7-bit mantissa, so
# exp(x) ~= bitcast_bf16(int16(A16*x + B16)), A16 = 2^7/log(2).
# A16 folds the 0.125 score scale; the -0.7*2^7/... shift centers the
# piecewise-linear error (classic magic-constant offset scaled to 2^7).
SCH_A16 = (128.0 / 0.6931471805599453) * 0.125
SCH_B16 = 16249.0

D = 1024
H = 16
DEP = 64
HID = 4096
B = 2
S = 2048
QLOC = 512
NCORES = 8
PADBIAS = -30000.0

P = 128
KT_D = D // P
MT_D = D // P
MT_H = HID // P
NPAIR = H // 2
VW = DEP + 1

PHASES = {}


def _mark(nc, name):
    PHASES[name] = nc.next_id()


# cpack column layout (f32, [P, CW]): widths per field
CFIELDS = [("bq", MT_D), ("bk", MT_D), ("bo", MT_D), ("b1", MT_H),
           ("b2", MT_D), ("a1", MT_D), ("be1", MT_D), ("a2", MT_D),
           ("be2", MT_D)]


def build(nkv: int, dve_ti: tuple = ()):
    assert nkv % P == 0
    nkt = nkv // P
    nchunk = (nkv + 511) // 512  # kv chunks of <=512

    cw = sum(w for _, w in CFIELDS) + nkt
    coff = {}
    off = 0
    for nm, w in CFIELDS:
        coff[nm] = off
        off += w
    coff["mb"] = off

    nc = bacc.Bacc(None, target_bir_lowering=False, debug=False)

    xq_d = nc.dram_tensor("xq", [P, D // P * QLOC], bf16, kind="ExternalInput")
    xq8_d = nc.dram_tensor("xq8", [P, D // P * QLOC], fp8, kind="ExternalInput")
    # xkv packed per kv-chunk: [P, nchunk * KT_D * chunkw]
    xkv_d = nc.dram_tensor("xkv", [P, KT_D * nkv], fp8, kind="ExternalInput")
    cpack_d = nc.dram_tensor("cpack", [P, cw], f32, kind="ExternalInput")
    wq_d = nc.dram_tensor("wq", [P, KT_D * D], fp8, kind="ExternalInput")
    wk_d = nc.dram_tensor("wk", [P, KT_D * D], fp8, kind="ExternalInput")
    wv_d = nc.dram_tensor("wv", [P, KT_D * D], fp8, kind="ExternalInput")
    wo_d = nc.dram_tensor("wo", [P, KT_D * D], bf16, kind="ExternalInput")
    w1_d = nc.dram_tensor("w1", [P, KT_D * HID], bf16, kind="ExternalInput")
    w2_d = nc.dram_tensor("w2", [P, MT_H * D], bf16, kind="ExternalInput")
    out_d = nc.dram_tensor("out", [D, QLOC], f32, kind="ExternalOutput")

    with tile.TileContext(nc) as tc, \
         nc.allow_low_precision(reason="bf16/f32r matmul inputs"), \
         ExitStack() as ctx:
        # ---- constants ----
        cst = ctx.enter_context(tc.tile_pool(name="cst", bufs=1))
        cpk = cst.tile([P, cw], f32)
        nc.sync.dma_start(out=cpk[:], in_=cpack_d[:])

        def ccol(nm, m=None, w=1):
            o = coff[nm]
            if m is None:
                return cpk[:, o:o + dict(CFIELDS)[nm]]
            return cpk[:, o + m:o + m + w]

        mbias = cpk[:, coff["mb"]:coff["mb"] + nkt]

        # ones tiles via memset + f32r rounding copy (no DMA)
        ones_f = cst.tile([P, 1], f32)
        nc.vector.memset(ones_f[:], 1.0)
        ones = cst.tile([P, 1], f32r)          # column of ones (LN sums lhsT)
        nc.vector.tensor_copy(ones[:], ones_f[:])
        scr = cst.tile([1, 1], f32)           # ACT table preload scratch
        # preload the exp table while the pipe is otherwise empty
        nc.scalar.activation(scr[:], cpk[0:1, 0:1], AF.Exp)

        # ---- x loads (ordered for fast Q start) ----
        es_x = ExitStack()
        p_xq = es_x.enter_context(tc.tile_pool(name="p_xq", bufs=1, side="right"))
        xqp = p_xq.tile([P, KT_D * QLOC], bf16, name="xqp")
        nc.sync.dma_start(out=xqp[:, 0:QLOC], in_=xq_d[:, 0:QLOC])
        xq = [xqp[:, k * QLOC:(k + 1) * QLOC] for k in range(KT_D)]

        es_w = ExitStack()
        wpool = es_w.enter_context(tc.tile_pool(name="wpool", bufs=2, side="right"))

        def load_whalf(dram, nm, half, split=False):
            t = wpool.tile([P, KT_D * 512], bf16, name=f"{nm}{half}", tag="w")
            base = half * 4096
            if split:
                nc.sync.dma_start(out=t[:, 0:512], in_=dram[:, base:base + 512])
                nc.sync.dma_start(out=t[:, 512:], in_=dram[:, base + 512:base + 4096])
            else:
                nc.sync.dma_start(out=t[:], in_=dram[:, base:base + 4096])
            return t

        # pools (left-SBUF creation order fixes LIFO release order)
        p_kt = ctx.enter_context(tc.tile_pool(name="p_kt", bufs=MT_D))
        p_qr = ctx.enter_context(tc.tile_pool(name="p_qr", bufs=MT_D))
        p_z = ctx.enter_context(tc.tile_pool(name="p_z", bufs=MT_D))
        w1p = ctx.enter_context(tc.tile_pool(name="w1p", bufs=2))
        ln_s = ctx.enter_context(tc.tile_pool(name="ln_s", bufs=2))
        es_attnT = ExitStack()
        p_attnT = es_attnT.enter_context(tc.tile_pool(name="p_attnT", bufs=MT_D))
        es_vaug = ExitStack()
        p_vaug = es_vaug.enter_context(tc.tile_pool(name="p_vaug", bufs=nkt))
        es_kv = ExitStack()
        p_xkv = es_kv.enter_context(tc.tile_pool(name="p_xkv", bufs=1))
        xkvp = p_xkv.tile([P, KT_D * nkv], fp8, name="xkvp")
        es_pp1 = ExitStack()
        pp1 = es_pp1.enter_context(
            tc.tile_pool(name="pp1", bufs=4, space="PSUM", side="right"))

        # ---- Q^T (fp8 DoubleRow) ----
        _mark(nc, 'qt')
        wq8 = wpool.tile([P, KT_D * D], fp8, name="wq8", tag="w")
        nc.sync.dma_start(out=wq8[:, 0:2048], in_=wq_d[:, 0:2048])
        xq8p = p_xq.tile([P, KT_D * QLOC], fp8, name="xq8p")
        nc.sync.dma_start(out=xq8p[:], in_=xq8_d[:])
        nc.sync.dma_start(out=wq8[:, 2048:], in_=wq_d[:, 2048:])
        nc.sync.dma_start(out=xqp[:, QLOC:], in_=xq_d[:, QLOC:])
        wq8r = wq8[:].rearrange("p (kk j m) -> p kk j m", j=2, m=D)
        xq8r = xq8p[:].rearrange("p (kk j c) -> p kk j c", j=2, c=QLOC)
        NKK = KT_D // 2

        qt = []
        for half in range(2):
            pss = [pp1.tile([P, QLOC], f32, name=f"qt_ps{ml}", tag="ps")
                   for ml in range(4)]
            for kk in range(NKK):
                for ml in range(4):
                    m = half * 4 + ml
                    nc.tensor.matmul(pss[ml][:],
                                     wq8r[:, kk, :, m * P:(m + 1) * P],
                                     xq8r[:, kk, :, :],
                                     start=(kk == 0), stop=(kk == NKK - 1),
                                     perf_mode=DR)
            for ml in range(4):
                m = half * 4 + ml
                t = p_qr.tile([P, QLOC], bf16, name=f"qt{m}", tag="qr")
                nc.vector.tensor_scalar(t[:], pss[ml][:], ccol("bq", m),
                                        1.0 / WSCALE, ALU.add, ALU.mult)
                qt.append(t)

        # ---- K^T (fp8 DoubleRow; weights pre-scaled x64 on host) ----
        _mark(nc, 'kt')
        wk8 = wpool.tile([P, KT_D * D], fp8, name="wk8", tag="w")
        nc.sync.dma_start(out=wk8[:], in_=wk_d[:])
        # xkv chunk loads: chunk 0 lands before the first K matmul needs it
        for c in range(nchunk):
            cwid = min(512, nkv - c * 512)
            nc.sync.dma_start(
                out=xkvp[:].rearrange("p (k c) -> p k c", c=nkv)[
                    :, :, c * 512:c * 512 + cwid],
                in_=xkv_d[:].rearrange("p (k c) -> p k c", c=nkv)[
                    :, :, c * 512:c * 512 + cwid])
        wk8r = wk8[:].rearrange("p (kk j m) -> p kk j m", j=2, m=D)
        xkv8r = xkvp[:].rearrange("p (kk j c) -> p kk j c", j=2, c=nkv)
        kt = []
        for m in range(MT_D):
            t = p_kt.tile([P, nkv], bf16, name=f"kt{m}", tag="kt")
            for c in range(nchunk):
                off = c * 512
                cwid = min(512, nkv - off)
                ps = pp1.tile([P, 512], f32, name="kt_ps", tag="ps")
                for kk in range(NKK):
                    nc.tensor.matmul(
                        ps[:, :cwid],
                        wk8r[:, kk, :, m * P:(m + 1) * P],
                        xkv8r[:, kk, :, off:off + cwid],
                        start=(kk == 0), stop=(kk == NKK - 1),
                        perf_mode=DR)
                nc.vector.tensor_scalar(t[:, off:off + cwid], ps[:, :cwid],
                                        ccol("bk", m), 1.0 / WSCALE,
                                        ALU.add, ALU.mult)
            kt.append(t)

        # ---- V (token-major, fp8 DoubleRow) with per-head ones column ----
        _mark(nc, 'v')
        wv8 = wpool.tile([P, KT_D * D], fp8, name="wv8", tag="w")
        nc.sync.dma_start(out=wv8[:], in_=wv_d[:])
        wv8r = wv8[:].rearrange("p (kk j m) -> p kk j m", j=2, m=D)
        vaug = []
        for ti in range(nkt):
            t = p_vaug.tile([P, H * VW], bf16, name=f"vaug{ti}", tag="vaug")
            v3 = t[:].rearrange("p (h c) -> p h c", c=VW)
            nc.gpsimd.memset(v3[:, :, DEP], 1.0)
            vaug.append(t)
        for half in range(2):
            for ti in range(nkt):
                ps = pp1.tile([P, 512], f32, name="v_ps", tag="ps")
                for kk in range(NKK):
                    nc.tensor.matmul(
                        ps[:], xkv8r[:, kk, :, ti * P:(ti + 1) * P],
                        wv8r[:, kk, :, half * 512:(half + 1) * 512],
                        start=(kk == 0), stop=(kk == NKK - 1),
                        perf_mode=DR)
                v3 = vaug[ti][:].rearrange("p (h c) -> p h c", c=VW)
                dst = v3[:, half * 8:(half + 1) * 8, 0:DEP]
                vsrc = ps[:].rearrange("p (h c) -> p h c", c=DEP)
                nc.scalar.activation(dst, vsrc, AF.Identity,
                                     scale=1.0 / WSCALE)
        es_kv.close()
        es_pp1.close()
        # V copies ran on ACT (Identity); swap the exp table back in
        # before the attention exps start
        nc.scalar.activation(scr[:], cpk[0:1, 0:1], AF.Exp)

        # ---- attention ----
        # The per-pair normalize (recip -> Pool partition-broadcast ->
        # multiply) is software-pipelined into the NEXT pair's kv loop so
        # PE never waits on it. op pool holds two pairs of output psums.
        _mark(nc, 'attn')
        ep = ExitStack()
        ep_sp = ExitStack()
        epl = ep.enter_context(tc.tile_pool(name="epl", bufs=3, side="right"))
        nrm = ep.enter_context(tc.tile_pool(name="nrm", bufs=2, side="right"))
        op = ep.enter_context(tc.tile_pool(name="op", bufs=2, space="PSUM"))
        sp = ep_sp.enter_context(tc.tile_pool(name="sp", bufs=3, space="PSUM"))
        attnT = [None] * NPAIR

        def fin_recs(st):
            st["recA"] = nrm.tile([1, QLOC], f32, name="recA", tag="rec")
            st["recB"] = nrm.tile([1, QLOC], f32, name="recB", tag="rec")
            nc.vector.reciprocal(st["recA"][:], st["psoA"][DEP:VW, :])
            nc.vector.reciprocal(st["recB"][:], st["psoB"][DEP:VW, :])

        def fin_bcast(st):
            st["rbA"] = nrm.tile([DEP, QLOC], f32, name="rbA", tag="rb")
            st["rbB"] = nrm.tile([DEP, QLOC], f32, name="rbB", tag="rb")
            nc.gpsimd.partition_broadcast(st["rbA"][:], st["recA"][:])
            nc.gpsimd.partition_broadcast(st["rbB"][:], st["recB"][:])

        def fin_mul(st):
            hp = st["hp"]
            at = p_attnT.tile([P, QLOC], bf16, name=f"attnT{hp}", tag="attnT")
            nc.vector.tensor_mul(at[0:DEP, :], st["psoA"][0:DEP, :],
                                 st["rbA"][:])
            tmpB = nrm.tile([DEP, QLOC], bf16, name="tmpB", tag="tmpB")
            nc.vector.tensor_mul(tmpB[:], st["psoB"][0:DEP, :], st["rbB"][:])
            nc.sync.dma_start(out=at[DEP:P, :], in_=tmpB[:])
            attnT[hp] = at

        prev = None
        for hp in range(NPAIR):
            hA, hB = 2 * hp, 2 * hp + 1
            psoA = op.tile([P, QLOC], f32, name="psoA", tag="pso")
            psoB = op.tile([P, QLOC], f32, name="psoB", tag="pso")

            def scores(ti):
                kvs = slice(ti * P, (ti + 1) * P)
                psAB = sp.tile([P, 2 * QLOC], f32, name="psAB", tag="sc")
                nc.tensor.matmul(psAB[:, 0:QLOC], kt[hp][0:DEP, kvs],
                                 qt[hp][0:DEP, :],
                                 start=True, stop=True, tile_position=(0, 0))
                nc.tensor.matmul(psAB[:, QLOC:2 * QLOC], kt[hp][DEP:P, kvs],
                                 qt[hp][DEP:P, :],
                                 start=True, stop=True, tile_position=(64, 0))
                if ti in dve_ti:
                    # Schraudolph fast exp on DVE, bf16-bits variant:
                    # bf16(exp(x)) ~= bitcast_bf16(int16(A16*x + B16))
                    ei = epl.tile([P, 2 * QLOC], i16, name="eABi", tag="e")
                    nc.vector.tensor_scalar(ei[:], psAB[:], SCH_A16, SCH_B16,
                                            ALU.mult, ALU.add)
                    return ("i", ei)
                eAB = epl.tile([P, 2 * QLOC], bf16, name="eAB", tag="e")
                nc.scalar.activation(eAB[:], psAB[:], AF.Exp,
                                     bias=mbias[:, ti:ti + 1], scale=0.125)
                return ("b", eAB)

            def eslice(e, lo, hi):
                tag, t = e
                ap = t[:, lo:hi]
                return ap.bitcast(bf16) if tag == "i" else ap

            eAB = scores(0)
            if prev is not None:
                fin_recs(prev)
            for ti in range(nkt):
                nxt = scores(ti + 1) if ti + 1 < nkt else None
                st, fi = (ti == 0), (ti == nkt - 1)
                nc.tensor.matmul(psoA[0:VW, :], vaug[ti][:, hA * VW:(hA + 1) * VW],
                                 eslice(eAB, 0, QLOC), start=st, stop=fi)
                nc.tensor.matmul(psoB[0:VW, :], vaug[ti][:, hB * VW:(hB + 1) * VW],
                                 eslice(eAB, QLOC, 2 * QLOC), start=st, stop=fi)
                if prev is not None:
                    if ti == 0:
                        fin_bcast(prev)
                    elif ti == nkt - 3:
                        fin_mul(prev)
                eAB = nxt
            prev = {"hp": hp, "psoA": psoA, "psoB": psoB}
            if hp == 0:
                # w1 group-0 preload rides under attention
                t = w1p.tile([P, KT_D * 1024], bf16, name="w1g0", tag="w1")
                nc.sync.dma_start(out=t[:], in_=w1_d[:, 0:KT_D * 1024])
                w1g_tiles = [t]
        ep_sp.close()  # free the scores psum banks before the normalize tail
        fin_recs(prev)
        fin_bcast(prev)
        fin_mul(prev)
        ep.close()
        es_vaug.close()

        # preload the sqrt table while ACT is otherwise idle (post-exp)
        nc.scalar.activation(scr[:], cpk[0:1, 0:1], AF.Sqrt)

        # ---- Wo + residual + interleaved LN1 stats ----
        _mark(nc, 'wo')
        wo0 = load_whalf(wo_d, "wo", 0)
        wo1 = load_whalf(wo_d, "wo", 1)
        pp2 = ctx.enter_context(
            tc.tile_pool(name="pp2", bufs=2, space="PSUM", side="right"))
        lnp = ctx.enter_context(
            tc.tile_pool(name="lnp", bufs=2, space="PSUM", side="right"))
        es_w1pp = ExitStack()
        w1pp = es_w1pp.enter_context(
            tc.tile_pool(name="w1pp", bufs=4, space="PSUM", side="right"))
        ssum1 = lnp.tile([1, QLOC], f32, name="ssum1", tag="lnps")
        ssq1 = lnp.tile([1, QLOC], f32, name="ssq1", tag="lnps")
        r1 = []
        for half in range(2):
            wo = wo0 if half == 0 else wo1
            for ml in range(4):
                m = half * 4 + ml
                ps = w1pp.tile([P, QLOC], f32, name="wo_ps", tag="w1ps")
                for k in range(KT_D):
                    nc.tensor.matmul(
                        ps[:], wo[:, k * 512 + ml * P:k * 512 + (ml + 1) * P],
                        attnT[k][:],
                        start=(k == 0), stop=(k == KT_D - 1))
                t = p_qr.tile([P, QLOC], f32r, name=f"r1_{m}", tag="qr")
                nc.vector.scalar_tensor_tensor(
                    t[:], ps[:], ccol("bo", m),
                    xqp[:, m * QLOC:(m + 1) * QLOC], ALU.add, ALU.add)
                r1.append(t)
                nc.tensor.matmul(ssum1[:], ones[:, 0:1], t[:],
                                 start=(m == 0), stop=(m == MT_D - 1))
                sq = ln_s.tile([P, QLOC], f32r, name="sq1", tag="sq", bufs=2)
                nc.vector.tensor_mul(sq[:], t[:].bitcast(f32),
                                     t[:].bitcast(f32))
                nc.tensor.matmul(ssq1[:], ones[:, 0:1], sq[:],
                                 start=(m == 0), stop=(m == MT_D - 1))
        es_w.close()
        es_x.close()
        es_attnT.close()

        def ln_head(ssum, ssq, tag):
            """Fused mean/var chain: returns (rstd, mrs) [1,QLOC] f32r."""
            n = D
            s1 = ln_s.tile([1, QLOC], f32, name=f"s1{tag}", tag="lns", bufs=7)
            nc.vector.tensor_copy(s1[:], ssum[:])
            t = ln_s.tile([1, QLOC], f32, name=f"t{tag}", tag="lns", bufs=7)
            nc.vector.scalar_tensor_tensor(t[:], s1[:], 1.0 / n, s1[:],
                                           ALU.mult, ALU.mult)
            vr = ln_s.tile([1, QLOC], f32, name=f"vr{tag}", tag="lns", bufs=7)
            nc.vector.tensor_sub(vr[:], ssq[:], t[:])
            std = ln_s.tile([1, QLOC], f32, name=f"std{tag}", tag="lns", bufs=7)
            nc.scalar.activation(std[:], vr[:], AF.Sqrt, scale=1.0 / (n - 1))
            rstd = ln_s.tile([1, QLOC], f32r, name=f"rstd{tag}", tag="lns", bufs=7)
            nc.vector.reciprocal(rstd[:], std[:])
            mrs = ln_s.tile([1, QLOC], f32r, name=f"mrs{tag}", tag="lns", bufs=7)
            nc.vector.scalar_tensor_tensor(mrs[:], s1[:], 1.0 / n,
                                           rstd[:].bitcast(f32),
                                           ALU.mult, ALU.mult)
            return rstd, mrs

        # ---- LN1 (normalize on DVE/Pool, alpha/beta on ACT; the rstd and
        # mean*rstd rows are partition-broadcast on Pool, no PSUM needed) ----
        _mark(nc, 'ln1')
        rstd1, mrs1 = ln_head(ssum1, ssq1, "1")
        out1 = [None] * MT_D
        bcs1 = ln_s.tile([P, 2 * QLOC], f32, name="bcs1", tag="lnb")
        rsb1s = bcs1[:, 0:QLOC]
        m2bs1 = bcs1[:, QLOC:]
        nc.gpsimd.partition_broadcast(rsb1s, rstd1[:].bitcast(f32))
        nc.gpsimd.partition_broadcast(m2bs1, mrs1[:].bitcast(f32))
        z1 = [None] * MT_D
        for m in range(MT_D):
            z = p_z.tile([P, QLOC], bf16, name=f"z1_{m}", tag="z")
            o = p_qr.tile([P, QLOC], f32, name=f"out1_{m}", tag="qr")
            eng = nc.gpsimd if m in (1, 3, 5, 7) else nc.vector
            tm = ln_s.tile([P, QLOC], f32, name="tm1", tag="tm", bufs=3)
            eng.tensor_mul(tm[:], r1[m][:].bitcast(f32), rsb1s)
            nc.vector.tensor_sub(z[:], tm[:], m2bs1)
            nc.scalar.activation(o[:], z[:], AF.Identity,
                                 bias=ccol("be1", m), scale=ccol("a1", m))
            z1[m] = z
            out1[m] = o

        def o1r(k):
            return z1[k][:]

        # ---- FFN first linear ----
        _mark(nc, 'w1')
        p_ht = ctx.enter_context(tc.tile_pool(name="p_ht", bufs=MT_H))
        ht = []
        # group 0 runs k-major so PE starts as soon as out1[k] tiles land
        for rnd in range(2):
            pss = [w1pp.tile([P, QLOC], f32, name=f"w1ps{rnd}_{mi}",
                             tag="w1ps") for mi in range(4)]
            for k in range(KT_D):
                for mi in range(4):
                    mm = rnd * 4 + mi
                    nc.tensor.matmul(
                        pss[mi][:],
                        w1g_tiles[0][:, k * 1024 + mm * P:
                                     k * 1024 + (mm + 1) * P],
                        o1r(k), start=(k == 0), stop=(k == KT_D - 1))
            for mi in range(4):
                mm = rnd * 4 + mi
                t = p_ht.tile([P, QLOC], bf16, name=f"ht{mm}", tag="ht")
                nc.scalar.activation(t[:], pss[mi][:], AF.Relu,
                                     bias=ccol("b1", mm))
                ht.append(t)
        es_w1pp.close()
        for g in range(1, 4):
            w1g = w1p.tile([P, KT_D * 1024], bf16, name=f"w1g{g}", tag="w1")
            nc.sync.dma_start(
                out=w1g[:], in_=w1_d[:, g * KT_D * 1024:(g + 1) * KT_D * 1024])
            for mm in range(8):
                m = g * 8 + mm
                ps = pp2.tile([P, QLOC], f32, name="h_ps", tag="ps2")
                for k in range(KT_D):
                    nc.tensor.matmul(
                        ps[:],
                        w1g[:, k * 1024 + mm * P:k * 1024 + (mm + 1) * P],
                        o1r(k),
                        start=(k == 0), stop=(k == KT_D - 1))
                t = p_ht.tile([P, QLOC], bf16, name=f"ht{m}", tag="ht")
                nc.scalar.activation(t[:], ps[:], AF.Relu,
                                     bias=ccol("b1", m))
                ht.append(t)

        # ---- FFN second linear + interleaved LN2 stats ----
        _mark(nc, 'w2')
        ssum2 = lnp.tile([1, QLOC], f32, name="ssum2", tag="lnps")
        ssq2 = lnp.tile([1, QLOC], f32, name="ssq2", tag="lnps")
        r2 = []
        w2p = ctx.enter_context(tc.tile_pool(name="w2p", bufs=2, side="right"))
        KH = MT_H // 2
        with tc.tile_pool(name="fpp", bufs=1, space="PSUM", side="right") as fpp:
            for mg in range(2):
                w2t = []
                for kh in range(2):
                    t = w2p.tile([P, KH * 512], bf16, name=f"w2q{mg}{kh}",
                                 tag="w2")
                    base = mg * MT_H * 512 + kh * KH * 512
                    nc.sync.dma_start(out=t[:],
                                      in_=w2_d[:, base:base + KH * 512])
                    w2t.append(t)
                f_ps = [fpp.tile([P, QLOC], f32, name=f"f_ps{mg}_{m}",
                                 tag=f"fps{m}", bufs=1) for m in range(4)]
                for k in range(MT_H):
                    wt = w2t[k // KH]
                    kk = k % KH
                    for m in range(4):
                        nc.tensor.matmul(
                            f_ps[m][:],
                            wt[:, kk * 512 + m * P:kk * 512 + (m + 1) * P],
                            ht[k][:],
                            start=(k == 0), stop=(k == MT_H - 1))
                for m in range(4):
                    mi = mg * 4 + m
                    t = p_kt.tile([P, QLOC], f32r, name=f"r2_{mi}", tag="kt")
                    nc.vector.scalar_tensor_tensor(t[:], f_ps[m][:],
                                                   ccol("b2", mi),
                                                   out1[mi][:], ALU.add, ALU.add)
                    r2.append(t)
                    nc.tensor.matmul(ssum2[:], ones[:, 0:1], t[:],
                                     start=(mi == 0), stop=(mi == MT_D - 1))
                    sq = ln_s.tile([P, QLOC], f32r, name="sq2", tag="sq", bufs=2)
                    nc.vector.tensor_mul(sq[:], t[:].bitcast(f32),
                                         t[:].bitcast(f32))
                    nc.tensor.matmul(ssq2[:], ones[:, 0:1], sq[:],
                                     start=(mi == 0), stop=(mi == MT_D - 1))

        # ---- LN2: normalize on DVE/Pool, alpha/beta on ACT, DMA per m ----
        _mark(nc, 'ln2')
        rstd2, mrs2 = ln_head(ssum2, ssq2, "2")
        bcs2 = ln_s.tile([P, 2 * QLOC], f32, name="bcs2", tag="lnb")
        rsb2s = bcs2[:, 0:QLOC]
        m2bs = bcs2[:, QLOC:]
        nc.gpsimd.partition_broadcast(rsb2s, rstd2[:].bitcast(f32))
        nc.gpsimd.partition_broadcast(m2bs, mrs2[:].bitcast(f32))
        for m in range(MT_D):
            o = ln_s.tile([P, QLOC], f32, name=f"ln2_{m}", tag="o2",
                          bufs=3)
            eng = nc.gpsimd if m in (3, 7) else nc.vector
            tm = ln_s.tile([P, QLOC], f32, name="tm2", tag="tm", bufs=3)
            eng.tensor_mul(tm[:], r2[m][:].bitcast(f32), rsb2s)
            tm2 = ln_s.tile([P, QLOC], f32, name="tq2", tag="tq", bufs=3)
            eng.tensor_sub(tm2[:], tm[:], m2bs)
            nc.scalar.activation(o[:], tm2[:], AF.Identity,
                                 bias=ccol("be2", m), scale=ccol("a2", m))
            nc.sync.dma_start(out=out_d[m * P:(m + 1) * P, :], in_=o[:])
        _mark(nc, 'end')

    nc.compile()
    return nc


_cache = {}


def _get_nc(nkv, dve_ti=()):
    key = (nkv, dve_ti)
    if key not in _cache:
        _cache[key] = build(nkv, dve_ti)
    return _cache[key]


def _pack_w(w, ncolblk):
    """[R, C] -> [128, (R//128)*C] with k-tiles of 128 rows as col blocks."""
    r, c = w.shape
    kt = r // P
    return np.ascontiguousarray(
        w.reshape(kt, P, c).transpose(1, 0, 2).reshape(P, kt * c))


def kernel(x, mask, Wq, bq, Wk, bk, Wv, bv, Wo, bo, alpha1, beta1,
           W1, b1, W2, b2, alpha2, beta2):
    x = np.asarray(x, np.float32)
    mask = np.asarray(mask)

    idx = [np.nonzero(np.asarray(mask[b]) == 0)[0] for b in range(B)]
    nkv = ((max(len(i) for i in idx) + P - 1) // P) * P
    nkv = max(nkv, P)
    nkt = nkv // P

    # kv tiles that are pad-free for every batch may use the DVE fast-exp;
    # interleave them (odd indices) so ACT and DVE exps overlap instead of
    # serializing in blocks on the scores-psum rotation.
    safe = min(min(len(i) for i in idx) // P, nkt)
    nd = min(nkt // 3, safe)
    dve_ti = tuple(range(safe - nd, safe))

    nc = _get_nc(nkv, dve_ti)

    def colmaj(v):
        v = np.asarray(v, np.float32)
        return v.reshape(-1, P).T

    bo_eff = (np.asarray(bo, np.float32)
              + np.asarray(bv, np.float32) @ np.asarray(Wo, np.float32))

    # LN1's alpha folds into W1 rows, beta into b1: the kernel feeds W1 the
    # pre-affine normalized activations.
    W1 = np.asarray(W1, np.float32)
    W1_eff = np.asarray(alpha1, np.float32)[:, None] * W1
    b1_eff = np.asarray(b1, np.float32) + np.asarray(beta1, np.float32) @ W1

    bf = ml_dtypes.bfloat16

    # packed constants
    fields = {"bq": colmaj(np.asarray(bq, np.float32) * WSCALE), "bk": colmaj(np.asarray(bk, np.float32) * WSCALE), "bo": colmaj(bo_eff),
              "b1": colmaj(b1_eff), "b2": colmaj(b2), "a1": colmaj(alpha1),
              "be1": colmaj(beta1), "a2": colmaj(alpha2), "be2": colmaj(beta2)}
    cw = sum(w for _, w in CFIELDS) + nkt

    # w2 packed per mg: [4096, 1024] -> mg slices of 512 cols, k-tiles packed
    W2f = np.asarray(W2, bf)
    w2pack = np.concatenate(
        [_pack_w(np.ascontiguousarray(W2f[:, mg * 512:(mg + 1) * 512]), 512)
         for mg in range(2)], axis=1)

    def _pack_blk(w, nblk, blkw):
        # [R, nblk*blkw] -> [128, nblk * (R//128) * blkw]:
        # layout [p, b*kt*blkw + k*blkw + col] = w[k*128+p, b*blkw+col]
        r = w.shape[0]
        kt = r // P
        return np.ascontiguousarray(
            w.reshape(kt, P, nblk, blkw).transpose(1, 2, 0, 3)
            .reshape(P, nblk * kt * blkw))

    e4 = mybir.dt.np(mybir.dt.float8e4)

    def _pack_dr(w):
        # [1024, C] -> [128, (kk=4, j=2, C)]: feature f = kk*256 + j*128 + p
        c = w.shape[1]
        return np.ascontiguousarray(
            w.reshape(4, 2, P, c).transpose(2, 0, 1, 3).reshape(P, 8 * c))

    common = {
        "wq": _pack_dr(np.asarray(Wq, np.float32) * WSCALE).astype(e4),
        "wk": _pack_dr(np.asarray(Wk, np.float32) * WSCALE).astype(e4),
        "wv": _pack_dr(np.asarray(Wv, np.float32) * WSCALE).astype(e4),
        "wo": _pack_blk(np.asarray(Wo, bf), 2, 512),
        "w1": _pack_blk(np.asarray(W1_eff, bf), 4, 1024),
        "w2": w2pack,
    }

    per_batch = []
    for b in range(B):
        ib = idx[b]
        xkv = np.zeros((D, nkv), np.float32)
        xkv[:, :len(ib)] = x[b][ib].T
        mb = np.zeros(nkv, np.float32)
        mb[len(ib):] = PADBIAS
        mb = np.ascontiguousarray(mb.reshape(nkt, P).T)
        cpk = np.zeros((P, cw), np.float32)
        off = 0
        for nm, w in CFIELDS:
            cpk[:, off:off + w] = fields[nm]
            off += w
        cpk[:, off:off + nkt] = mb
        per_batch.append((_pack_dr(xkv).astype(e4), np.ascontiguousarray(cpk),
                          np.ascontiguousarray(x[b].T)))

    in_maps = []
    for c in range(NCORES):
        b = c // 4
        qoff = (c % 4) * QLOC
        xkvp, cpk, xT = per_batch[b]
        xq_blk = xT[:, qoff:qoff + QLOC]
        m = dict(common)
        m["xq"] = _pack_w(np.ascontiguousarray(xq_blk.astype(bf)), QLOC)
        m["xq8"] = _pack_dr(np.ascontiguousarray(xq_blk)).astype(e4)
        m["xkv"] = xkvp
        m["cpack"] = cpk
        in_maps.append(m)

    res = None
    for attempt in range(3):
        try:
            res = run_bass_kernel_spmd(nc, in_maps, list(range(NCORES)))
            break
        except Exception:
            if attempt == 2:
                raise

    out = np.empty((B, S, D), np.float32)
    for c in range(NCORES):
        b = c // 4
        qoff = (c % 4) * QLOC
        out[b, qoff:qoff + QLOC, :] = res.results[c]["out"].T
    return out

